# revision 20
# baseline (speedup 1.0000x reference)
"""2-layer LSTM (B=1024, T=256, I=64, H=128) + FC head on 8 NeuronCores.

Data-parallel: batch sharded 8 ways (128 rows/core), weights replicated.
On-chip orientation keeps state transposed (hT: [H partitions, B free]) so the
recurrence matmuls, activations and cell updates never need a transpose.

v3 design:
- fp16 matmul operands (1 cycle/row on PE), fp32 PSUM accumulate.
- Gate order (i, f, o, g); g preacts carry 2x from weight prep so
  tanh(g) = 2*sig(2g) - 1 comes out of one fused sigmoid with i,f,o.
- Cell state kept as ct = c/2 so the update is ct = t1 + t2 with
  t1 = (sig2g - 0.5)*sig_i  (= sig_i*tanh(g)/2)     [DVE stt]
  t2 = sig_f * ct                                   [DVE tt, fp16 2x]
  ct = t1 + t2                                      [DVE tt, fp16 2x]
- tanh(c) via sigmoid: sg2 = sig(4*ct) = sig(2c) on the Act engine
  (scale immediate, same act table as the gate sigmoid).
- h state kept as h'' = sig(2c)*sig_o = h/2 + sig_o/2 [DVE tt, fp16 2x].
  Every consumer of h uses weights 2W (absorbs h/2) plus a correction
  matmul with weights -W against rhs sig_o, accumulated in PSUM.
- Emission is phase-ordered per iteration so the layer-0 recurrence loop
  (MM -> sig1 -> cell -> sig2 -> h'' -> MM) never queues behind layer-1
  work on the in-order engines.
"""

import numpy as np

B, T, I, H = 1024, 256, 64, 128
NCORES = 8
BC = B // NCORES  # 128 batch rows per core
XCHUNK = 32  # timesteps per staged x DMA chunk
C16 = True  # keep ct = c/2 in fp16 (DVE 2x); False -> fp32

_cache = {}


def _build(has_b1, has_bfc, nsteps):
    import concourse.bacc as bacc
    import concourse.tile as tile
    import concourse.mybir as mybir

    f32 = mybir.dt.float32
    mdt = mybir.dt.float16
    cdt = mdt if C16 else f32
    Alu = mybir.AluOpType
    Act = mybir.ActivationFunctionType

    nc = bacc.Bacc("TRN2", target_bir_lowering=False, debug=False)

    xt_d = nc.dram_tensor("xt", [I + 2, nsteps, BC], mdt, kind="ExternalInput")
    w0x_d = nc.dram_tensor("w0x", [4, I + 2, H], mdt, kind="ExternalInput")
    # *c variants are the -W/2 correction weights for the sig_o rhs
    w0h_d = nc.dram_tensor("w0h", [4, H, H], mdt, kind="ExternalInput")
    w0hc_d = nc.dram_tensor("w0hc", [4, H, H], mdt, kind="ExternalInput")
    w1x_d = nc.dram_tensor("w1x", [4, H, H], mdt, kind="ExternalInput")
    w1xc_d = nc.dram_tensor("w1xc", [4, H, H], mdt, kind="ExternalInput")
    w1h_d = nc.dram_tensor("w1h", [4, H, H], mdt, kind="ExternalInput")
    w1hc_d = nc.dram_tensor("w1hc", [4, H, H], mdt, kind="ExternalInput")
    wfc_d = nc.dram_tensor("wfc", [H, 2], mdt, kind="ExternalInput")
    b1_d = nc.dram_tensor("b1", [4, 1, H], mdt, kind="ExternalInput") if has_b1 else None
    bfc_d = nc.dram_tensor("bfc", [1, 1], mdt, kind="ExternalInput") if has_bfc else None
    out_d = nc.dram_tensor("out", [1, BC], f32, kind="ExternalOutput")

    with tile.TileContext(nc) as tc:
        with (
            tc.tile_pool(name="singles", bufs=1) as singles,
            tc.tile_pool(name="sg0", bufs=4) as sg0p,
            tc.tile_pool(name="sg1", bufs=4) as sg1p,
            tc.tile_pool(name="sg2", bufs=2) as sg2p,
            tc.tile_pool(name="tmp", bufs=2) as tmpp,
            tc.tile_pool(name="ps0", bufs=3, space="PSUM") as ps0p,
            tc.tile_pool(name="ps1", bufs=3, space="PSUM") as ps1p,
            tc.tile_pool(name="psfc", bufs=1, space="PSUM") as psfc,
        ):
            xta = xt_d.ap()
            nchunk = (nsteps + XCHUNK - 1) // XCHUNK
            xts = []
            for j in range(nchunk):
                t0, t1 = j * XCHUNK, min((j + 1) * XCHUNK, nsteps)
                xt_t = singles.tile([I + 2, (t1 - t0) * BC], mdt, tag=f"xt{j}", name=f"xt{j}")
                nc.sync.dma_start(
                    out=xt_t[:], in_=xta[:, t0:t1, :].rearrange("p t b -> p (t b)")
                )
                xts.append(xt_t)

            def load_w(dram, k, q, tag):
                w = singles.tile([k, H], mdt, tag=f"{tag}{q}", name=f"{tag}{q}")
                nc.sync.dma_start(out=w[:], in_=dram.ap()[q])
                return w

            w0x = [load_w(w0x_d, I + 2, q, "w0x") for q in range(4)]
            w0h = [load_w(w0h_d, H, q, "w0h") for q in range(4)]
            w0hc = [load_w(w0hc_d, H, q, "w0hc") for q in range(4)]
            w1x = [load_w(w1x_d, H, q, "w1x") for q in range(4)]
            w1xc = [load_w(w1xc_d, H, q, "w1xc") for q in range(4)]
            w1h = [load_w(w1h_d, H, q, "w1h") for q in range(4)]
            w1hc = [load_w(w1hc_d, H, q, "w1hc") for q in range(4)]
            wfc = singles.tile([H, 2], mdt, tag="wfc", name="wfc")
            nc.sync.dma_start(out=wfc[:], in_=wfc_d.ap())
            b1 = None
            ones = None
            if has_b1 or has_bfc:
                ones = singles.tile([1, BC], mdt, tag="ones", name="ones")
                nc.vector.memset(ones[:], 1.0)
            if has_b1:
                b1 = [load_w(b1_d, 1, q, "b1") for q in range(4)]
            bfc = None
            if has_bfc:
                bfc = singles.tile([1, 1], mdt, tag="bfc", name="bfc")
                nc.sync.dma_start(out=bfc[:], in_=bfc_d.ap())

            cs = []
            for layer in range(2):
                c = singles.tile([H, BC], cdt, tag=f"c{layer}", name=f"c{layer}")
                nc.vector.memset(c[:], 0.0)
                cs.append(c)
            RING = 6
            rings = [
                [singles.tile([H, BC], mdt, tag=f"h{layer}r{s}", name=f"h{layer}r{s}") for s in range(RING)]
                for layer in range(2)
            ]
            # sig_o slices of recent sg tiles, keyed [layer][t] (kept alive by
            # the sg pools' bufs=3 ring; consumers lag at most 2 iterations)
            sgo = [{}, {}]

            # --- matmul emission helpers -------------------------------------
            # PSUM discipline: one accumulation group per bank ([H, 4*BC] tile):
            # start=True ONLY on the first-emitted matmul of the bank (it marks
            # the whole 2KB zero-region pending-zero; later first-writes
            # overwrite, subsequent writes accumulate), stop=True ONLY on the
            # last. Tile's bank-level WAW keeps execution in emission order.
            def mm_l0(t, ps):
                """All matmuls for layer-0 step 0 into ps (gate blocks)."""
                for q in range(4):
                    sl = ps[:, q * BC : (q + 1) * BC]
                    j, r = t // XCHUNK, t % XCHUNK
                    rhs = xts[j][:, r * BC : (r + 1) * BC]
                    nc.tensor.matmul(sl, w0x[q][:], rhs, start=(q == 0), stop=(q == 3))

            def mm_l0_pre(t, ps):
                """x-projection + sig_o correction for layer-0 step t (ready mid-iteration)."""
                for q in range(4):
                    sl = ps[:, q * BC : (q + 1) * BC]
                    j, r = t // XCHUNK, t % XCHUNK
                    rhs = xts[j][:, r * BC : (r + 1) * BC]
                    nc.tensor.matmul(sl, w0x[q][:], rhs, start=(q == 0), stop=False)
                    nc.tensor.matmul(sl, w0hc[q][:], sgo[0][t - 1], start=False, stop=False)

            def mm_l0_post(t, ps):
                """h-projection for layer-0 step t (needs h0''(t-1))."""
                for q in range(4):
                    sl = ps[:, q * BC : (q + 1) * BC]
                    nc.tensor.matmul(sl, w0h[q][:], rings[0][(t - 1) % RING][:],
                                     start=False, stop=(q == 3))

            def mm_l1_pre(t, ps):
                """Layer-1 step t matmuls not needing h1''(t-1): corrections + xproj."""
                first = True
                for q in range(4):
                    sl = ps[:, q * BC : (q + 1) * BC]
                    if t > 0:
                        nc.tensor.matmul(sl, w1hc[q][:], sgo[1][t - 1], start=first, stop=False)
                        first = False
                    nc.tensor.matmul(sl, w1xc[q][:], sgo[0][t], start=first, stop=False)
                    first = False
                    last = t == 0 and q == 3 and not has_b1
                    nc.tensor.matmul(sl, w1x[q][:], rings[0][t % RING][:], start=False, stop=last)
                    if has_b1:
                        nc.tensor.matmul(sl, b1[q][:], ones[:], start=False,
                                         stop=(t == 0 and q == 3))

            def mm_l1_post(t, ps):
                """Layer-1 h-projection for step t (needs h1''(t-1))."""
                if t == 0:
                    return
                for q in range(4):
                    sl = ps[:, q * BC : (q + 1) * BC]
                    nc.tensor.matmul(sl, w1h[q][:], rings[1][(t - 1) % RING][:],
                                     start=False, stop=(q == 3))

            # --- elementwise phase helpers -----------------------------------
            def sig1(layer, t, ps):
                pool = sg0p if layer == 0 else sg1p
                sg = pool.tile([H, 4 * BC], mdt, tag=f"sg{layer}", name=f"sg{layer}_{t}")
                nc.scalar.activation(sg[:], ps[:], Act.Sigmoid)
                sgo[layer][t] = sg[:, 2 * BC : 3 * BC]
                sgo[layer].pop(t - 4, None)
                return sg

            def cell(layer, t, sg):
                t1_ = tmpp.tile([H, BC], mdt, tag=f"t1_{layer}", name=f"t1_{layer}_{t}")
                # (sig2g - 0.5) * sig_i  == 0.5 * sig_i * tanh(g_pre)
                nc.vector.scalar_tensor_tensor(
                    t1_[:], sg[:, 3 * BC : 4 * BC], 0.5, sg[:, 0:BC],
                    Alu.subtract, Alu.mult,
                )
                t2_ = tmpp.tile([H, BC], cdt, tag=f"t2_{layer}", name=f"t2_{layer}_{t}")
                nc.vector.tensor_tensor(t2_[:], sg[:, BC : 2 * BC], cs[layer][:], Alu.mult)
                # ct = c/2 = t1 + t2 ; layer 1's update runs on Pool so its
                # sig2 lands after the loop-critical sig1_0 on the Act engine
                eng = nc.vector if layer == 0 else nc.gpsimd
                eng.tensor_tensor(cs[layer][:], t1_[:], t2_[:], Alu.add)

            def sig2(layer, t):
                sg2 = sg2p.tile([H, BC], mdt, tag=f"sg2_{layer}", name=f"sg2_{layer}_{t}")
                nc.scalar.activation(sg2[:], cs[layer][:], Act.Sigmoid, scale=4.0)
                return sg2

            def hout(layer, t, sg2):
                # h'' = sig(2c) * sig_o = h/2 + sig_o/2
                h = rings[layer][t % RING]
                nc.vector.tensor_tensor(h[:], sg2[:], sgo[layer][t], Alu.mult)

            # --- schedule ----------------------------------------------------
            # Phase order per iteration keeps the layer-0 recurrence loop
            # (post0 -> sig1_0 -> cell0 -> sig2_0 -> h0'' -> post0) earliest
            # in priority at every readiness tie; layer-1 trails by ~1 step.
            # iteration 0: layer-0 step 0 only
            ps0 = ps0p.tile([H, 4 * BC], f32, tag="ps0", name="ps0_0")
            mm_l0(0, ps0)
            sgA = sig1(0, 0, ps0)
            cell(0, 0, sgA)
            s2A = sig2(0, 0)
            hout(0, 0, s2A)
            if nsteps > 1:
                ps0n = ps0p.tile([H, 4 * BC], f32, tag="ps0", name="ps0_1")
                mm_l0_pre(1, ps0n)
                mm_l0_post(1, ps0n)
                ps0 = ps0n
            ps1 = ps1p.tile([H, 4 * BC], f32, tag="ps1", name="ps1_0")
            mm_l1_pre(0, ps1)
            mm_l1_post(0, ps1)

            for t in range(1, nsteps):
                sgA = sig1(0, t, ps0)
                sgB = sig1(1, t - 1, ps1)
                cell(0, t, sgA)
                s2A = sig2(0, t)
                hout(0, t, s2A)
                if t + 1 < nsteps:
                    ps0n = ps0p.tile([H, 4 * BC], f32, tag="ps0", name=f"ps0_{t+1}")
                    mm_l0_pre(t + 1, ps0n)
                    mm_l0_post(t + 1, ps0n)
                    ps0 = ps0n
                cell(1, t - 1, sgB)
                s2B = sig2(1, t - 1)
                hout(1, t - 1, s2B)
                ps1 = ps1p.tile([H, 4 * BC], f32, tag="ps1", name=f"ps1_{t}")
                mm_l1_pre(t, ps1)
                mm_l1_post(t, ps1)

            # tail: layer-1 step nsteps-1
            tl = nsteps - 1
            sgB = sig1(1, tl, ps1)
            cell(1, tl, sgB)
            s2B = sig2(1, tl)
            hout(1, tl, s2B)

            # FC head: out = 2*Wfc . h1'' + (-Wfc) . sig_o1  (+ bfc)
            pf = psfc.tile([1, BC], f32, tag="fc", name="fc")
            nc.tensor.matmul(pf[:], wfc[:, 0:1], rings[1][tl % RING][:], start=True, stop=False)
            nc.tensor.matmul(pf[:], wfc[:, 1:2], sgo[1][tl], start=False, stop=not has_bfc)
            if has_bfc:
                nc.tensor.matmul(pf[:], bfc[:], ones[:], start=False, stop=True)
            ot = singles.tile([1, BC], f32, tag="ot", name="ot")
            nc.vector.tensor_copy(ot[:], pf[:])
            nc.sync.dma_start(out=out_d.ap(), in_=ot[:])

    nc.compile()
    return nc


def _prep_weights(Wih, Whh, b, in_dim, fold_bias, h_scale):
    """Repack [4H, in] PyTorch-gate-order (i,f,g,o) weights into per-gate
    lhsT tiles [in(+1), H] with gate order (i,f,o,g), g scaled by 2 (sigmoid
    trick), and an extra h_scale on weights whose input is h-like."""
    order = [0, 1, 3, 2]  # i, f, o, g
    pad = 2 if fold_bias else 0
    wx = np.zeros((4, in_dim + pad, H), np.float32)
    wh = np.empty((4, H, H), np.float32)
    for qi, q in enumerate(order):
        scale = 2.0 if q == 2 else 1.0
        wx[qi, :in_dim] = (Wih[q * H : (q + 1) * H] * (scale * h_scale[0])).T
        if fold_bias:
            wx[qi, in_dim] = b[q * H : (q + 1) * H] * scale
        wh[qi] = (Whh[q * H : (q + 1) * H] * (scale * h_scale[1])).T
    return wx, wh


def kernel(x, Wih0, Whh0, b0, Wih1, Whh1, b1, Wfc, bfc, _nsteps=T):
    from concourse.bass_utils import run_bass_kernel_spmd

    x = np.asarray(x, np.float32)
    nsteps = _nsteps
    has_b1 = bool(np.any(np.asarray(b1)))
    has_bfc = bool(np.any(np.asarray(bfc)))

    # consumers of h get 2W (for h''=h/2+sig_o/2) plus -W correction vs sig_o
    w0x, w0h = _prep_weights(np.asarray(Wih0, np.float32), np.asarray(Whh0, np.float32),
                             np.asarray(b0, np.float32), I, True, (1.0, 2.0))
    w1x, w1h = _prep_weights(np.asarray(Wih1, np.float32), np.asarray(Whh1, np.float32),
                             np.asarray(b1, np.float32), H, False, (2.0, 2.0))
    wfc2 = np.asarray(Wfc, np.float32).reshape(1, H).T * 2.0   # [H,1]
    wfcc = -0.5 * wfc2
    wfc = np.ascontiguousarray(np.concatenate([wfc2, wfcc], axis=1))  # [H,2]

    key = (has_b1, has_bfc, nsteps)
    if key not in _cache:
        _cache[key] = _build(has_b1, has_bfc, nsteps)
    nc = _cache[key]

    mnp = np.float16
    in_maps = []
    for c in range(NCORES):
        xc = x[c * BC : (c + 1) * BC, :nsteps]  # [BC, t, I]
        xt = np.zeros((I + 2, nsteps, BC), np.float32)
        xt[:I] = xc.transpose(2, 1, 0)
        xt[I] = 1.0
        m = {"xt": xt.astype(mnp),
             "w0x": w0x.astype(mnp),
             "w0h": w0h.astype(mnp), "w0hc": (-0.5 * w0h).astype(mnp),
             "w1x": w1x.astype(mnp), "w1xc": (-0.5 * w1x).astype(mnp),
             "w1h": w1h.astype(mnp), "w1hc": (-0.5 * w1h).astype(mnp),
             "wfc": wfc.astype(mnp)}
        if has_b1:
            border = [0, 1, 3, 2]
            bb = np.empty((4, 1, H), np.float32)
            for qi, q in enumerate(border):
                bb[qi, 0] = np.asarray(b1, np.float32)[q * H : (q + 1) * H] * (2.0 if q == 2 else 1.0)
            m["b1"] = bb.astype(mnp)
        if has_bfc:
            m["bfc"] = np.asarray(bfc, np.float32).reshape(1, 1).astype(mnp)
        in_maps.append(m)

    res = run_bass_kernel_spmd(nc, in_maps, list(range(NCORES)))
    globals()["LAST_RESULT"] = res
    globals()["LAST_RUN"] = (nc, in_maps)
    out = np.empty((B, 1), np.float32)
    for c in range(NCORES):
        out[c * BC : (c + 1) * BC, 0] = res.results[c]["out"][0]
    return out


def bench(iters=6):
    """Re-run the last compiled kernel, returning per-call wall seconds."""
    import time
    from concourse.bass_utils import run_bass_kernel_spmd

    nc, in_maps = globals()["LAST_RUN"]
    times = []
    for _ in range(iters):
        t0 = time.perf_counter()
        run_bass_kernel_spmd(nc, in_maps, list(range(NCORES)))
        times.append(time.perf_counter() - t0)
    return times
